# revision 13
# baseline (speedup 1.0000x reference)
"""Trainium2 Bass kernel for CoordinationMemory (scatter_memory).

Computation (per batch row n):
    cur_h = memory[n, veh_idx[n], :]
    x     = concat(veh_repr[n], cust_repr[n], edge_emb[n])        # [3D]
    nh    = tanh(x @ W_in + b_in + cur_h @ W_h + b_h)             # [H]
    out   = memory with out[n, veh_idx[n], :] = nh

Full shapes: N=4096, L_V=64, H=512, D=256. Data-parallel over 8 cores
(512 rows each).

The output is memory with only 512 of 32768 rows rewritten, so the
dominant cost of a naive kernel is the 64 MiB/core DRAM->DRAM copy of the
untouched rows (~375 us at the ~358 GB/s HBM roofline). This kernel
eliminates the copy: the per-core memory shard is DONATED as the
ExternalOutput buffer (XLA input-output aliasing, the same donation
mechanism run_bass_via_pjrt uses for its zero-initialized outputs), so
the NEFF sees `out` pre-populated with the memory contents and only has
to gather the 512 active rows, run the small GEMMs, and scatter the 512
updated rows back in place (~6 MiB of HBM traffic total).

run_bass_kernel_spmd's axon redirect hardcodes zero-filled donated
outputs, so the dispatch below inlines the same shard_map/_bass_exec_p
path with the memory shard in the donated slot instead.

GEMMs run on the PE in float32r (1 cycle/row at >=256-wide moving dim,
4x faster than plain fp32) with fp32 PSUM accumulation. The biases are
applied with a K=1 ones-vector matmul into the same PSUM accumulation.
"""

import numpy as np
import jax
from jax.sharding import Mesh, PartitionSpec

from jax.experimental.shard_map import shard_map

import concourse.bass as bass
import concourse.tile as tile
from concourse import bacc, mybir
from concourse.bass2jax import (
    _bass_exec_p,
    install_neuronx_cc_hook,
    partition_id_tensor,
)
from concourse.masks import make_identity

N = 4096
LV = 64
H = 512
D = 256
NCORES = 8
NS = N // NCORES          # rows per core
P = 128
NT = NS // P              # 4 row-tiles per core
KC = (3 * D) // P         # 6 contraction chunks for x @ W_in
HC = H // P               # 4 contraction chunks for cur_h @ W_h

F32 = mybir.dt.float32
F32R = mybir.dt.float32r
I32 = mybir.dt.int32

IN_NAMES = ("xt", "wtop", "wh", "bsum", "idx")


def build_program(repeats=1, do_consts=True, do_gath=True, do_gemm=True, do_scat=True):
    nc = bacc.Bacc(
        "TRN2",
        target_bir_lowering=False,
        debug=False,
        enable_asserts=False,
        num_devices=NCORES,
    )
    xt = nc.dram_tensor("xt", (NT, KC, P, P), F32R, kind="ExternalInput").ap()
    wtop = nc.dram_tensor("wtop", (KC, P, H), F32R, kind="ExternalInput").ap()
    wh = nc.dram_tensor("wh", (HC, P, H), F32R, kind="ExternalInput").ap()
    bsum = nc.dram_tensor("bsum", (1, H), F32R, kind="ExternalInput").ap()
    # idx[p, t] = (t*128 + p)*LV + veh_idx[t*128 + p]  (absolute row in out_flat)
    idx = nc.dram_tensor("idx", (P, NT), I32, kind="ExternalInput").ap()
    out = nc.dram_tensor("out", (NS, LV, H), F32, kind="ExternalOutput").ap()
    out_flat = out.rearrange("n l h -> (n l) h")

    with tile.TileContext(nc) as tc:
        with (
            tc.tile_pool(name="const", bufs=1) as constp,
            tc.tile_pool(name="wts", bufs=2) as wtsp,
            tc.tile_pool(name="gath", bufs=NT + 1) as gathp,
            tc.tile_pool(name="work", bufs=2) as workp,
            tc.tile_pool(name="stage", bufs=2) as stagep,
            tc.tile_pool(name="psum", bufs=2, space="PSUM") as psump,
            tc.tile_pool(name="psumtr", bufs=4, space="PSUM") as psumtrp,
        ):
            ident = constp.tile([P, P], F32)
            make_identity(nc, ident[:])
            ones_f32 = constp.tile([1, P], F32)
            nc.vector.memset(ones_f32[:], 1.0)
            ones = constp.tile([1, P], F32R)
            nc.vector.tensor_copy(out=ones[:], in_=ones_f32[:])

            def body():
                idx_sb = stagep.tile([P, NT], I32)
                nc.sync.dma_start(out=idx_sb[:], in_=idx[:])

                # Constants balanced across the two HWDGE rings, ordered so
                # that what gates the first GEMMs arrives first:
                #   SP  (sync):   idx, xt_t0, wtop
                #   ACT (scalar): bsum, wh, xt_t1..t3
                xt_sbs = [wtsp.tile([P, KC * P], F32R, name=f"xt_sb{t}") for t in range(NT)]
                wtop_sb = wtsp.tile([P, KC * H], F32R)
                wh_sb = wtsp.tile([P, HC * H], F32R)
                bs_sb = wtsp.tile([1, H], F32R)
                if do_consts:
                    nc.scalar.dma_start(out=bs_sb[:], in_=bsum[:])
                    for c in range(KC):
                        nc.sync.dma_start(
                            out=xt_sbs[0][:, bass.ts(c, P)], in_=xt[0, c]
                        )
                    for c in range(HC):
                        nc.scalar.dma_start(out=wh_sb[:, bass.ts(c, H)], in_=wh[c])
                    for c in range(KC):
                        nc.sync.dma_start(out=wtop_sb[:, bass.ts(c, H)], in_=wtop[c])
                    for t in range(1, NT):
                        for c in range(KC):
                            nc.scalar.dma_start(
                                out=xt_sbs[t][:, bass.ts(c, P)], in_=xt[t, c]
                            )
                else:
                    # tiny sliver loads: keep tiles written (allocator) and
                    # dep structure intact while removing ~99% of the traffic
                    nc.scalar.dma_start(out=bs_sb[:], in_=bsum[:])
                    for t in range(NT):
                        for c in range(KC):
                            nc.sync.dma_start(
                                out=xt_sbs[t][:, c * P : c * P + 4], in_=xt[t, c][:, :4]
                            )
                    for c in range(KC):
                        nc.sync.dma_start(out=wtop_sb[:, c * H : c * H + 4], in_=wtop[c][:, :4])
                    for c in range(HC):
                        nc.scalar.dma_start(out=wh_sb[:, c * H : c * H + 4], in_=wh[c][:, :4])

                # All gathers up front: they read out_flat, which every
                # scatter below writes; issuing them first keeps the
                # conservative whole-tensor deps from serializing
                # gather_{t+1} behind scatter_t.
                cur_hs = []
                for t in range(NT):
                    cur_h = gathp.tile([P, H], F32)
                    if do_gath:
                        nc.gpsimd.indirect_dma_start(
                            out=cur_h[:],
                            out_offset=None,
                            in_=out_flat[:],
                            in_offset=bass.IndirectOffsetOnAxis(
                                ap=idx_sb[:, t : t + 1], axis=0
                            ),
                        )
                    cur_hs.append(cur_h)

                for t in range(NT):
                    nh = stagep.tile([P, H], F32)
                    if not do_gemm:
                        nc.vector.tensor_copy(out=nh[:], in_=cur_hs[t][:])
                    if do_gemm:
                        # cur_h [n, h] -> cur_hT [h, n] in 128x128 blocks via PE.
                        cur_ht = workp.tile([P, H], F32R)
                        for b in range(HC):
                            ptr = psumtrp.tile([P, P], F32, space="PSUM")
                            nc.tensor.transpose(
                                out=ptr[:],
                                in_=cur_hs[t][:, bass.ts(b, P)],
                                identity=ident[:],
                            )
                            nc.vector.tensor_copy(
                                out=cur_ht[:, bass.ts(b, P)], in_=ptr[:]
                            )

                        pmm = psump.tile([P, H], F32, space="PSUM")
                        # bias: ones^T @ (b_in + b_h) broadcasts the bias row.
                        # Order: bias, wh terms, then xt terms - wtop chunks
                        # stream in last, so they're consumed last.
                        nc.tensor.matmul(
                            out=pmm[:],
                            lhsT=ones[:],
                            rhs=bs_sb[:],
                            start=True,
                            stop=False,
                        )
                        for b in range(HC):
                            nc.tensor.matmul(
                                out=pmm[:],
                                lhsT=cur_ht[:, bass.ts(b, P)],
                                rhs=wh_sb[:, bass.ts(b, H)],
                                start=False,
                                stop=False,
                            )
                        for c in range(KC):
                            nc.tensor.matmul(
                                out=pmm[:],
                                lhsT=xt_sbs[t][:, bass.ts(c, P)],
                                rhs=wtop_sb[:, bass.ts(c, H)],
                                start=False,
                                stop=(c == KC - 1),
                            )

                        nc.scalar.activation(
                            out=nh[:],
                            in_=pmm[:],
                            func=mybir.ActivationFunctionType.Tanh,
                        )
                    if do_scat:
                        nc.gpsimd.indirect_dma_start(
                            out=out_flat[:],
                            out_offset=bass.IndirectOffsetOnAxis(
                                ap=idx_sb[:, t : t + 1], axis=0
                            ),
                            in_=nh[:],
                            in_offset=None,
                        )

            if repeats == 1:
                body()
            else:
                with tc.For_i(0, repeats, 1):
                    body()

    nc.compile()
    return nc


def build_dispatch(nc, donate=True):
    """jit'd shard_map callable over 8 cores; arg order IN_NAMES + mem
    (donated as the `out` buffer). Returns fn(xt, wtop, wh, bsum, idx, mem)
    -> (out,) with global (axis-0 concatenated) arrays."""
    install_neuronx_cc_hook()
    pname = nc.partition_id_tensor.name if nc.partition_id_tensor else None
    in_names = list(IN_NAMES) + ["out"] + ([pname] if pname else [])
    out_avals = (jax.core.ShapedArray((NS, LV, H), np.float32),)

    def _body(*args):
        ops = list(args)
        if pname:
            ops.append(partition_id_tensor())
        outs = _bass_exec_p.bind(
            *ops,
            out_avals=out_avals,
            in_names=tuple(in_names),
            out_names=("out",),
            lowering_input_output_aliases=(),
            sim_require_finite=True,
            sim_require_nnan=True,
            nc=nc,
        )
        return tuple(outs)

    devices = jax.devices()[:NCORES]
    assert len(devices) == NCORES, f"need {NCORES} cores, have {len(jax.devices())}"
    mesh = Mesh(np.asarray(devices), ("core",))
    nargs = len(IN_NAMES) + 1
    return jax.jit(
        shard_map(
            _body,
            mesh=mesh,
            in_specs=(PartitionSpec("core"),) * nargs,
            out_specs=(PartitionSpec("core"),),
            check_rep=False,
        ),
        donate_argnums=(nargs - 1,) if donate else (),
        keep_unused=True,
    )


def make_global_inputs(
    memory, veh_idx, veh_repr, cust_repr, edge_emb, W_in, b_in, W_h, b_h
):
    """Host-side prep: global (8*per-core axis 0) arrays in IN_NAMES order + mem."""
    mem = np.ascontiguousarray(np.asarray(memory, dtype=np.float32))
    veh = np.asarray(veh_idx).astype(np.int64).reshape(N)
    x_cat = np.concatenate(
        (
            np.asarray(veh_repr, dtype=np.float32)[:, 0, :],
            np.asarray(cust_repr, dtype=np.float32)[:, 0, :],
            np.asarray(edge_emb, dtype=np.float32)[:, 0, 0, :],
        ),
        axis=1,
    )  # [N, 768]
    # xt[core, t, c, p, n] = x_cat[core*NS + t*128 + n, c*128 + p]
    xtg = np.ascontiguousarray(
        x_cat.reshape(NCORES, NT, P, KC, P).transpose(0, 1, 3, 4, 2)
    ).reshape(NCORES * NT, KC, P, P)
    wtopg = np.ascontiguousarray(
        np.broadcast_to(
            np.asarray(W_in, dtype=np.float32).reshape(1, KC, P, H),
            (NCORES, KC, P, H),
        )
    ).reshape(NCORES * KC, P, H)
    whg = np.ascontiguousarray(
        np.broadcast_to(
            np.asarray(W_h, dtype=np.float32).reshape(1, HC, P, H),
            (NCORES, HC, P, H),
        )
    ).reshape(NCORES * HC, P, H)
    bsumg = np.ascontiguousarray(
        np.broadcast_to(
            (np.asarray(b_in, dtype=np.float32) + np.asarray(b_h, dtype=np.float32))[
                None, :
            ],
            (NCORES, H),
        )
    )
    # idx[core, p, t] = (t*128 + p)*LV + veh[core*NS + t*128 + p]
    base = (np.arange(NT)[None, :, None] * P + np.arange(P)[None, None, :]) * LV
    idxg = (
        (base + veh.reshape(NCORES, NT, P))
        .transpose(0, 2, 1)
        .reshape(NCORES * P, NT)
        .astype(np.int32)
    )
    return [xtg, wtopg, whg, bsumg, idxg, mem]


_PROGRAM = None
_FN = None


def _get_fn():
    global _PROGRAM, _FN
    if _FN is None:
        _PROGRAM = build_program()
        _FN = build_dispatch(_PROGRAM)
    return _FN


def kernel(memory, veh_idx, veh_repr, cust_repr, edge_emb, W_in, b_in, W_h, b_h):
    fn = _get_fn()
    args = make_global_inputs(
        memory, veh_idx, veh_repr, cust_repr, edge_emb, W_in, b_in, W_h, b_h
    )
    (out,) = fn(*args)
    return np.asarray(out)


# revision 15
# speedup vs baseline: 112.4436x; 112.4436x over previous
"""Trainium2 Bass kernel for CoordinationMemory (scatter_memory).

Computation (per batch row n):
    cur_h = memory[n, veh_idx[n], :]
    x     = concat(veh_repr[n], cust_repr[n], edge_emb[n])        # [3D]
    nh    = tanh(x @ W_in + b_in + cur_h @ W_h + b_h)             # [H]
    out   = memory with out[n, veh_idx[n], :] = nh

Full shapes: N=4096, L_V=64, H=512, D=256. Data-parallel over 8 cores
(512 rows each).

The output is memory with only 512 of 32768 rows rewritten, so the
dominant cost of a naive kernel is the 64 MiB/core DRAM->DRAM copy of the
untouched rows (~375 us at the ~358 GB/s HBM roofline). This kernel
eliminates the copy: the per-core memory shard is DONATED as the
ExternalOutput buffers (XLA input-output aliasing, the same donation
mechanism run_bass_via_pjrt uses for its zero-initialized outputs), so
the NEFF sees the outputs pre-populated with the memory contents and only
has to gather the 512 active rows, run the small GEMMs, and scatter the
512 updated rows back in place (~6 MiB of HBM traffic total).

run_bass_kernel_spmd's axon redirect hardcodes zero-filled donated
outputs, so the dispatch below inlines the same shard_map/_bass_exec_p
path with the memory shards in the donated slots instead.

Structure notes:
- The output is split into one tensor per 128-row tile so the indirect
  gather/scatter of different tiles (which Tile tracks as whole-tensor
  accesses) don't falsely serialize. The four tensors get DISTINCT shapes
  (identical flat layout) so XLA's donation pairing is forced 1:1.
- GEMMs run on the PE in float32r (1 cycle/row at >=256-wide moving dim,
  4x faster than plain fp32) with fp32 PSUM accumulation; inputs are
  pre-laid-out on the host so each SBUF tile loads with ONE large DMA.
- Biases are applied with a K=1 ones-vector matmul into the same PSUM
  accumulation group.
"""

import numpy as np
import jax
from jax.sharding import Mesh, PartitionSpec

from jax.experimental.shard_map import shard_map

import concourse.bass as bass
import concourse.tile as tile
from concourse import bacc, mybir
from concourse.bass2jax import (
    _bass_exec_p,
    install_neuronx_cc_hook,
    partition_id_tensor,
)
from concourse.masks import make_identity

N = 4096
LV = 64
H = 512
D = 256
NCORES = 8
NS = N // NCORES          # rows per core
P = 128
NT = NS // P              # 4 row-tiles per core
KC = (3 * D) // P         # 6 contraction chunks for x @ W_in
HC = H // P               # 4 contraction chunks for cur_h @ W_h

F32 = mybir.dt.float32
F32R = mybir.dt.float32r
I32 = mybir.dt.int32

IN_NAMES = ("xt", "wtop", "wh", "bsum", "idx")
# distinct shapes (same flat layout) force 1:1 donation pairing
OUT_SHAPES = [(P, LV, H), (P // 2, 2 * LV, H), (P // 4, 4 * LV, H), (P // 8, 8 * LV, H)]


def build_program(repeats=1, do_consts=True, do_gath=True, do_gemm=True, do_scat=True):
    nc = bacc.Bacc(
        "TRN2",
        target_bir_lowering=False,
        debug=False,
        enable_asserts=False,
        num_devices=NCORES,
    )
    # xt[t]: [P(k in chunk), KC*P] per-partition-contiguous x^T for tile t
    xt = nc.dram_tensor("xt", (NT, P, KC * P), F32R, kind="ExternalInput").ap()
    # wtop: [P(k in chunk), KC*H], wtop[p, c*H+h] = W_in[c*P+p, h]
    wtop = nc.dram_tensor("wtop", (P, KC * H), F32R, kind="ExternalInput").ap()
    wh = nc.dram_tensor("wh", (P, HC * H), F32R, kind="ExternalInput").ap()
    bsum = nc.dram_tensor("bsum", (1, H), F32R, kind="ExternalInput").ap()
    # idx[p, t] = p*LV + veh_idx[t*128 + p]  (row in out_t's flat view)
    idx = nc.dram_tensor("idx", (P, NT), I32, kind="ExternalInput").ap()
    outs = [
        nc.dram_tensor(f"out{t}", OUT_SHAPES[t], F32, kind="ExternalOutput").ap()
        for t in range(NT)
    ]
    out_flats = [o.rearrange("a b h -> (a b) h") for o in outs]

    with tile.TileContext(nc) as tc:
        with (
            tc.tile_pool(name="const", bufs=1) as constp,
            tc.tile_pool(name="wts", bufs=2) as wtsp,
            tc.tile_pool(name="gath", bufs=NT + 1) as gathp,
            tc.tile_pool(name="work", bufs=2) as workp,
            tc.tile_pool(name="stage", bufs=2) as stagep,
            tc.tile_pool(name="psum", bufs=2, space="PSUM") as psump,
            tc.tile_pool(name="psumtr", bufs=4, space="PSUM") as psumtrp,
        ):
            ident = constp.tile([P, P], F32)
            make_identity(nc, ident[:])
            ones_f32 = constp.tile([1, P], F32)
            nc.vector.memset(ones_f32[:], 1.0)
            ones = constp.tile([1, P], F32R)
            nc.vector.tensor_copy(out=ones[:], in_=ones_f32[:])

            def body():
                idx_sb = stagep.tile([P, NT], I32)
                nc.sync.dma_start(out=idx_sb[:], in_=idx[:])

                # 8 coalesced const loads, balanced across the two HWDGE
                # rings, ordered so what gates the first GEMMs arrives first:
                #   SP  (sync):   idx, xt0, xt1, wtop
                #   ACT (scalar): bsum, wh, xt2, xt3
                xt_sbs = [
                    wtsp.tile([P, KC * P], F32R, name=f"xt_sb{t}") for t in range(NT)
                ]
                wtop_sb = wtsp.tile([P, KC * H], F32R)
                wh_sb = wtsp.tile([P, HC * H], F32R)
                bs_sb = wtsp.tile([1, H], F32R)
                if do_consts:
                    nc.scalar.dma_start(out=bs_sb[:], in_=bsum[:])
                    nc.sync.dma_start(out=xt_sbs[0][:], in_=xt[0])
                    nc.scalar.dma_start(out=wh_sb[:], in_=wh[:])
                    nc.sync.dma_start(out=xt_sbs[1][:], in_=xt[1])
                    nc.scalar.dma_start(out=xt_sbs[2][:], in_=xt[2])
                    nc.sync.dma_start(out=wtop_sb[:], in_=wtop[:])
                    nc.scalar.dma_start(out=xt_sbs[3][:], in_=xt[3])
                else:
                    # tiny sliver loads keep tiles written (allocator) and
                    # dep structure intact with ~1% of the traffic
                    nc.scalar.dma_start(out=bs_sb[:], in_=bsum[:])
                    for t in range(NT):
                        nc.sync.dma_start(out=xt_sbs[t][:, :4], in_=xt[t][:, :4])
                    nc.sync.dma_start(out=wtop_sb[:, :4], in_=wtop[:, :4])
                    nc.scalar.dma_start(out=wh_sb[:, :4], in_=wh[:, :4])

                cur_hs = []
                for t in range(NT):
                    cur_h = gathp.tile([P, H], F32, name=f"cur_h{t}")
                    if do_gath:
                        nc.gpsimd.indirect_dma_start(
                            out=cur_h[:],
                            out_offset=None,
                            in_=out_flats[t][:],
                            in_offset=bass.IndirectOffsetOnAxis(
                                ap=idx_sb[:, t : t + 1], axis=0
                            ),
                        )
                    cur_hs.append(cur_h)

                for t in range(NT):
                    nh = stagep.tile([P, H], F32)
                    if not do_gemm:
                        nc.vector.tensor_copy(out=nh[:], in_=cur_hs[t][:])
                    if do_gemm:
                        # cur_h [n, h] -> cur_hT [h, n] in 128x128 blocks via PE
                        cur_ht = workp.tile([P, H], F32R)
                        for b in range(HC):
                            ptr = psumtrp.tile([P, P], F32, space="PSUM")
                            nc.tensor.transpose(
                                out=ptr[:],
                                in_=cur_hs[t][:, bass.ts(b, P)],
                                identity=ident[:],
                            )
                            nc.vector.tensor_copy(
                                out=cur_ht[:, bass.ts(b, P)], in_=ptr[:]
                            )

                        pmm = psump.tile([P, H], F32, space="PSUM")
                        # bias via K=1 ones matmul; then wh terms; xt terms
                        # last (wtop streams in latest on the SP ring)
                        nc.tensor.matmul(
                            out=pmm[:],
                            lhsT=ones[:],
                            rhs=bs_sb[:],
                            start=True,
                            stop=False,
                        )
                        for b in range(HC):
                            nc.tensor.matmul(
                                out=pmm[:],
                                lhsT=cur_ht[:, bass.ts(b, P)],
                                rhs=wh_sb[:, bass.ts(b, H)],
                                start=False,
                                stop=False,
                            )
                        for c in range(KC):
                            nc.tensor.matmul(
                                out=pmm[:],
                                lhsT=xt_sbs[t][:, bass.ts(c, P)],
                                rhs=wtop_sb[:, bass.ts(c, H)],
                                start=False,
                                stop=(c == KC - 1),
                            )

                        nc.scalar.activation(
                            out=nh[:],
                            in_=pmm[:],
                            func=mybir.ActivationFunctionType.Tanh,
                        )
                    if do_scat:
                        nc.gpsimd.indirect_dma_start(
                            out=out_flats[t][:],
                            out_offset=bass.IndirectOffsetOnAxis(
                                ap=idx_sb[:, t : t + 1], axis=0
                            ),
                            in_=nh[:],
                            in_offset=None,
                        )

            if repeats == 1:
                body()
            else:
                with tc.For_i(0, repeats, 1):
                    body()

    nc.compile()
    return nc


def build_dispatch(nc, donate=True):
    """jit'd shard_map callable over 8 cores; arg order IN_NAMES + mem tiles
    (donated as the out0..out3 buffers). Returns fn(xt, wtop, wh, bsum, idx,
    m0, m1, m2, m3) -> (out0..out3,) with global (axis-0 concat) arrays."""
    install_neuronx_cc_hook()
    pname = nc.partition_id_tensor.name if nc.partition_id_tensor else None
    out_names = [f"out{t}" for t in range(NT)]
    in_names = list(IN_NAMES) + out_names + ([pname] if pname else [])
    out_avals = tuple(
        jax.core.ShapedArray(OUT_SHAPES[t], np.float32) for t in range(NT)
    )

    def _body(*args):
        ops = list(args)
        if pname:
            ops.append(partition_id_tensor())
        outs = _bass_exec_p.bind(
            *ops,
            out_avals=out_avals,
            in_names=tuple(in_names),
            out_names=tuple(out_names),
            lowering_input_output_aliases=(),
            sim_require_finite=True,
            sim_require_nnan=True,
            nc=nc,
        )
        return tuple(outs)

    devices = jax.devices()[:NCORES]
    assert len(devices) == NCORES, f"need {NCORES} cores, have {len(jax.devices())}"
    mesh = Mesh(np.asarray(devices), ("core",))
    nargs = len(IN_NAMES) + NT
    return jax.jit(
        shard_map(
            _body,
            mesh=mesh,
            in_specs=(PartitionSpec("core"),) * nargs,
            out_specs=(PartitionSpec("core"),) * NT,
            check_rep=False,
        ),
        donate_argnums=tuple(range(len(IN_NAMES), nargs)) if donate else (),
        keep_unused=True,
    )


def make_global_inputs(
    memory, veh_idx, veh_repr, cust_repr, edge_emb, W_in, b_in, W_h, b_h
):
    """Host-side prep: global (8*per-core axis 0) arrays, IN_NAMES order +
    the four per-tile memory slices (donated as out0..out3)."""
    mem = np.ascontiguousarray(np.asarray(memory, dtype=np.float32))
    veh = np.asarray(veh_idx).astype(np.int64).reshape(N)
    x_cat = np.concatenate(
        (
            np.asarray(veh_repr, dtype=np.float32)[:, 0, :],
            np.asarray(cust_repr, dtype=np.float32)[:, 0, :],
            np.asarray(edge_emb, dtype=np.float32)[:, 0, 0, :],
        ),
        axis=1,
    )  # [N, 768]
    # xt[core*NT+t, p, c*P+n] = x_cat[(core*NT+t)*P + n, c*P + p]
    xtg = np.ascontiguousarray(
        x_cat.reshape(NCORES * NT, P, KC, P).transpose(0, 3, 2, 1)
    ).reshape(NCORES * NT, P, KC * P)
    w_in = np.asarray(W_in, dtype=np.float32)  # [KC*P, H]
    # wtop[p, c*H+h] = W_in[c*P+p, h]
    wtop1 = np.ascontiguousarray(w_in.reshape(KC, P, H).transpose(1, 0, 2)).reshape(
        1, P, KC * H
    )
    wtopg = np.ascontiguousarray(np.broadcast_to(wtop1, (NCORES, P, KC * H))).reshape(
        NCORES * P, KC * H
    )
    w_h = np.asarray(W_h, dtype=np.float32)  # [HC*P, H]
    wh1 = np.ascontiguousarray(w_h.reshape(HC, P, H).transpose(1, 0, 2)).reshape(
        1, P, HC * H
    )
    whg = np.ascontiguousarray(np.broadcast_to(wh1, (NCORES, P, HC * H))).reshape(
        NCORES * P, HC * H
    )
    bsumg = np.ascontiguousarray(
        np.broadcast_to(
            (np.asarray(b_in, dtype=np.float32) + np.asarray(b_h, dtype=np.float32))[
                None, :
            ],
            (NCORES, H),
        )
    )
    # idx[core, p, t] = p*LV + veh[core*NS + t*P + p]
    base = np.arange(P, dtype=np.int64)[None, None, :] * LV  # [1, 1, P]
    idxg = (
        (base + veh.reshape(NCORES, NT, P))
        .transpose(0, 2, 1)
        .reshape(NCORES * P, NT)
        .astype(np.int32)
    )
    # memory tile slices, reshaped to the distinct out shapes
    mem_tiles = []
    m = mem.reshape(NCORES, NT, P, LV, H)
    for t in range(NT):
        s = OUT_SHAPES[t]
        mem_tiles.append(np.ascontiguousarray(m[:, t]).reshape(NCORES * s[0], *s[1:]))
    return [xtg, wtopg, whg, bsumg, idxg] + mem_tiles


_PROGRAM = None
_FN = None


def _get_fn():
    global _PROGRAM, _FN
    if _FN is None:
        _PROGRAM = build_program()
        _FN = build_dispatch(_PROGRAM)
    return _FN


def kernel(memory, veh_idx, veh_repr, cust_repr, edge_emb, W_in, b_in, W_h, b_h):
    fn = _get_fn()
    args = make_global_inputs(
        memory, veh_idx, veh_repr, cust_repr, edge_emb, W_in, b_in, W_h, b_h
    )
    outs = fn(*args)
    # outs[t] global: (NCORES*s0, ...) -> per-core (P, LV, H) tiles -> full
    full = np.empty((NCORES, NT, P, LV, H), np.float32)
    for t, o in enumerate(outs):
        full[:, t] = np.asarray(o).reshape(NCORES, P, LV, H)
    return full.reshape(N, LV, H)


# revision 17
# speedup vs baseline: 141.1658x; 1.2554x over previous
"""Trainium2 Bass kernel for CoordinationMemory (scatter_memory).

Computation (per batch row n):
    cur_h = memory[n, veh_idx[n], :]
    x     = concat(veh_repr[n], cust_repr[n], edge_emb[n])        # [3D]
    nh    = tanh(x @ W_in + b_in + cur_h @ W_h + b_h)             # [H]
    out   = memory with out[n, veh_idx[n], :] = nh

Full shapes: N=4096, L_V=64, H=512, D=256. Data-parallel over 8 cores
(512 rows each).

The output is memory with only 512 of 32768 rows rewritten, so the
dominant cost of a naive kernel is the 64 MiB/core DRAM->DRAM copy of the
untouched rows (~375 us at the ~358 GB/s HBM roofline). This kernel
eliminates the copy: the per-core memory shard is DONATED as the
ExternalOutput buffers (XLA input-output aliasing, the same donation
mechanism run_bass_via_pjrt uses for its zero-initialized outputs), so
the NEFF sees the outputs pre-populated with the memory contents and only
has to gather the 512 active rows, run the small GEMMs, and scatter the
512 updated rows back in place (~6 MiB of HBM traffic total).

run_bass_kernel_spmd's axon redirect hardcodes zero-filled donated
outputs, so the dispatch below inlines the same shard_map/_bass_exec_p
path with the memory shards in the donated slots instead.

Structure notes:
- The output is split into one tensor per 128-row tile so the indirect
  gather/scatter of different tiles (which Tile tracks as whole-tensor
  accesses) don't falsely serialize. The four tensors get DISTINCT shapes
  (identical flat layout) so XLA's donation pairing is forced 1:1.
- GEMMs run on the PE in float32r (1 cycle/row at >=256-wide moving dim,
  4x faster than plain fp32) with fp32 PSUM accumulation; inputs are
  pre-laid-out on the host so each SBUF tile loads with ONE large DMA.
- Biases are applied with a K=1 ones-vector matmul into the same PSUM
  accumulation group.
"""

import numpy as np
import ml_dtypes
import jax
from jax.sharding import Mesh, PartitionSpec

from jax.experimental.shard_map import shard_map

import concourse.bass as bass
import concourse.tile as tile
from concourse import bacc, mybir
from concourse.bass2jax import (
    _bass_exec_p,
    install_neuronx_cc_hook,
    partition_id_tensor,
)
from concourse.masks import make_identity

N = 4096
LV = 64
H = 512
D = 256
NCORES = 8
NS = N // NCORES          # rows per core
P = 128
NT = NS // P              # 4 row-tiles per core
KC = (3 * D) // P         # 6 contraction chunks for x @ W_in
HC = H // P               # 4 contraction chunks for cur_h @ W_h

F32 = mybir.dt.float32
F32R = mybir.dt.float32r
BF16 = mybir.dt.bfloat16
I32 = mybir.dt.int32

IN_NAMES = ("xt", "wtop", "wh", "bsum", "idx")
# distinct shapes (same flat layout) force 1:1 donation pairing
OUT_SHAPES = [(P, LV, H), (P // 2, 2 * LV, H), (P // 4, 4 * LV, H), (P // 8, 8 * LV, H)]


def build_program(repeats=1, do_consts=True, do_gath=True, do_gemm=True, do_scat=True):
    nc = bacc.Bacc(
        "TRN2",
        target_bir_lowering=False,
        debug=False,
        enable_asserts=False,
        num_devices=NCORES,
    )
    # xt[t]: [P(k in chunk), KC*P] per-partition-contiguous x^T for tile t
    xt = nc.dram_tensor("xt", (NT, P, KC * P), BF16, kind="ExternalInput").ap()
    # wtop: [P(k in chunk), KC*H], wtop[p, c*H+h] = W_in[c*P+p, h]
    wtop = nc.dram_tensor("wtop", (P, KC * H), BF16, kind="ExternalInput").ap()
    wh = nc.dram_tensor("wh", (P, HC * H), BF16, kind="ExternalInput").ap()
    bsum = nc.dram_tensor("bsum", (1, H), BF16, kind="ExternalInput").ap()
    # idx[p, t] = p*LV + veh_idx[t*128 + p]  (row in out_t's flat view)
    idx = nc.dram_tensor("idx", (P, NT), I32, kind="ExternalInput").ap()
    outs = [
        nc.dram_tensor(f"out{t}", OUT_SHAPES[t], F32, kind="ExternalOutput").ap()
        for t in range(NT)
    ]
    out_flats = [o.rearrange("a b h -> (a b) h") for o in outs]

    with tile.TileContext(nc) as tc:
        with (
            tc.tile_pool(name="const", bufs=1) as constp,
            tc.tile_pool(name="wts", bufs=2) as wtsp,
            tc.tile_pool(name="gath", bufs=NT + 1) as gathp,
            tc.tile_pool(name="work", bufs=2) as workp,
            tc.tile_pool(name="stage", bufs=2) as stagep,
            tc.tile_pool(name="psum", bufs=2, space="PSUM") as psump,
            tc.tile_pool(name="psumtr", bufs=4, space="PSUM") as psumtrp,
        ):
            ident = constp.tile([P, P], F32)
            make_identity(nc, ident[:])
            ones_f32 = constp.tile([1, P], F32)
            nc.vector.memset(ones_f32[:], 1.0)
            ones = constp.tile([1, P], BF16)
            nc.vector.tensor_copy(out=ones[:], in_=ones_f32[:])

            def body():
                idx_sb = stagep.tile([P, NT], I32)
                nc.sync.dma_start(out=idx_sb[:], in_=idx[:])

                # 8 coalesced const loads, balanced across the two HWDGE
                # rings, ordered so what gates the first GEMMs arrives first:
                #   SP  (sync):   idx, xt0, xt1, wtop
                #   ACT (scalar): bsum, wh, xt2, xt3
                xt_sbs = [
                    wtsp.tile([P, KC * P], BF16, name=f"xt_sb{t}") for t in range(NT)
                ]
                wtop_sb = wtsp.tile([P, KC * H], BF16)
                wh_sb = wtsp.tile([P, HC * H], BF16)
                bs_sb = wtsp.tile([1, H], BF16)
                if do_consts:
                    nc.scalar.dma_start(out=bs_sb[:], in_=bsum[:])
                    nc.sync.dma_start(out=xt_sbs[0][:], in_=xt[0])
                    nc.scalar.dma_start(out=wh_sb[:], in_=wh[:])
                    nc.sync.dma_start(out=xt_sbs[1][:], in_=xt[1])
                    nc.scalar.dma_start(out=xt_sbs[2][:], in_=xt[2])
                    nc.sync.dma_start(out=wtop_sb[:], in_=wtop[:])
                    nc.scalar.dma_start(out=xt_sbs[3][:], in_=xt[3])
                else:
                    # tiny sliver loads keep tiles written (allocator) and
                    # dep structure intact with ~1% of the traffic
                    nc.scalar.dma_start(out=bs_sb[:], in_=bsum[:])
                    for t in range(NT):
                        nc.sync.dma_start(out=xt_sbs[t][:, :4], in_=xt[t][:, :4])
                    nc.sync.dma_start(out=wtop_sb[:, :4], in_=wtop[:, :4])
                    nc.scalar.dma_start(out=wh_sb[:, :4], in_=wh[:, :4])

                cur_hs = []
                for t in range(NT):
                    cur_h = gathp.tile([P, H], F32, name=f"cur_h{t}")
                    if do_gath:
                        nc.gpsimd.indirect_dma_start(
                            out=cur_h[:],
                            out_offset=None,
                            in_=out_flats[t][:],
                            in_offset=bass.IndirectOffsetOnAxis(
                                ap=idx_sb[:, t : t + 1], axis=0
                            ),
                        )
                    else:
                        nc.vector.memset(cur_h[:], 0.0)
                    cur_hs.append(cur_h)

                for t in range(NT):
                    nh = stagep.tile([P, H], F32)
                    if not do_gemm:
                        nc.vector.tensor_copy(out=nh[:], in_=cur_hs[t][:])
                    if do_gemm:
                        # cur_h [n, h] -> cur_hT [h, n] in 128x128 blocks via PE
                        cur_ht = workp.tile([P, H], BF16)
                        for b in range(HC):
                            ptr = psumtrp.tile([P, P], F32, space="PSUM")
                            nc.tensor.transpose(
                                out=ptr[:],
                                in_=cur_hs[t][:, bass.ts(b, P)],
                                identity=ident[:],
                            )
                            nc.vector.tensor_copy(
                                out=cur_ht[:, bass.ts(b, P)], in_=ptr[:]
                            )

                        pmm = psump.tile([P, H], F32, space="PSUM")
                        # bias via K=1 ones matmul; then wh terms; xt terms
                        # last (wtop streams in latest on the SP ring)
                        nc.tensor.matmul(
                            out=pmm[:],
                            lhsT=ones[:],
                            rhs=bs_sb[:],
                            start=True,
                            stop=False,
                        )
                        for b in range(HC):
                            nc.tensor.matmul(
                                out=pmm[:],
                                lhsT=cur_ht[:, bass.ts(b, P)],
                                rhs=wh_sb[:, bass.ts(b, H)],
                                start=False,
                                stop=False,
                            )
                        for c in range(KC):
                            nc.tensor.matmul(
                                out=pmm[:],
                                lhsT=xt_sbs[t][:, bass.ts(c, P)],
                                rhs=wtop_sb[:, bass.ts(c, H)],
                                start=False,
                                stop=(c == KC - 1),
                            )

                        nc.scalar.activation(
                            out=nh[:],
                            in_=pmm[:],
                            func=mybir.ActivationFunctionType.Tanh,
                        )
                    if do_scat:
                        nc.gpsimd.indirect_dma_start(
                            out=out_flats[t][:],
                            out_offset=bass.IndirectOffsetOnAxis(
                                ap=idx_sb[:, t : t + 1], axis=0
                            ),
                            in_=nh[:],
                            in_offset=None,
                        )

            if repeats == 1:
                body()
            else:
                with tc.For_i(0, repeats, 1):
                    body()

    nc.compile()
    return nc


def build_dispatch(nc, donate=True):
    """jit'd shard_map callable over 8 cores; arg order IN_NAMES + mem tiles
    (donated as the out0..out3 buffers). Returns fn(xt, wtop, wh, bsum, idx,
    m0, m1, m2, m3) -> (out0..out3,) with global (axis-0 concat) arrays."""
    install_neuronx_cc_hook()
    pname = nc.partition_id_tensor.name if nc.partition_id_tensor else None
    out_names = [f"out{t}" for t in range(NT)]
    in_names = list(IN_NAMES) + out_names + ([pname] if pname else [])
    out_avals = tuple(
        jax.core.ShapedArray(OUT_SHAPES[t], np.float32) for t in range(NT)
    )

    def _body(*args):
        ops = list(args)
        if pname:
            ops.append(partition_id_tensor())
        outs = _bass_exec_p.bind(
            *ops,
            out_avals=out_avals,
            in_names=tuple(in_names),
            out_names=tuple(out_names),
            lowering_input_output_aliases=(),
            sim_require_finite=True,
            sim_require_nnan=True,
            nc=nc,
        )
        return tuple(outs)

    devices = jax.devices()[:NCORES]
    assert len(devices) == NCORES, f"need {NCORES} cores, have {len(jax.devices())}"
    mesh = Mesh(np.asarray(devices), ("core",))
    nargs = len(IN_NAMES) + NT
    return jax.jit(
        shard_map(
            _body,
            mesh=mesh,
            in_specs=(PartitionSpec("core"),) * nargs,
            out_specs=(PartitionSpec("core"),) * NT,
            check_rep=False,
        ),
        donate_argnums=tuple(range(len(IN_NAMES), nargs)) if donate else (),
        keep_unused=True,
    )


def make_global_inputs(
    memory, veh_idx, veh_repr, cust_repr, edge_emb, W_in, b_in, W_h, b_h
):
    """Host-side prep: global (8*per-core axis 0) arrays, IN_NAMES order +
    the four per-tile memory slices (donated as out0..out3)."""
    mem = np.ascontiguousarray(np.asarray(memory, dtype=np.float32))
    veh = np.asarray(veh_idx).astype(np.int64).reshape(N)
    x_cat = np.concatenate(
        (
            np.asarray(veh_repr, dtype=np.float32)[:, 0, :],
            np.asarray(cust_repr, dtype=np.float32)[:, 0, :],
            np.asarray(edge_emb, dtype=np.float32)[:, 0, 0, :],
        ),
        axis=1,
    )  # [N, 768]
    # xt[core*NT+t, p, c*P+n] = x_cat[(core*NT+t)*P + n, c*P + p]
    xtg = np.ascontiguousarray(
        x_cat.reshape(NCORES * NT, P, KC, P).transpose(0, 3, 2, 1)
    ).reshape(NCORES * NT, P, KC * P).astype(ml_dtypes.bfloat16)
    w_in = np.asarray(W_in, dtype=np.float32)  # [KC*P, H]
    # wtop[p, c*H+h] = W_in[c*P+p, h]
    wtop1 = np.ascontiguousarray(w_in.reshape(KC, P, H).transpose(1, 0, 2)).reshape(
        1, P, KC * H
    )
    wtopg = np.ascontiguousarray(
        np.broadcast_to(wtop1.astype(ml_dtypes.bfloat16), (NCORES, P, KC * H))
    ).reshape(NCORES * P, KC * H)
    w_h = np.asarray(W_h, dtype=np.float32)  # [HC*P, H]
    wh1 = np.ascontiguousarray(w_h.reshape(HC, P, H).transpose(1, 0, 2)).reshape(
        1, P, HC * H
    )
    whg = np.ascontiguousarray(
        np.broadcast_to(wh1.astype(ml_dtypes.bfloat16), (NCORES, P, HC * H))
    ).reshape(NCORES * P, HC * H)
    bsumg = np.ascontiguousarray(
        np.broadcast_to(
            (np.asarray(b_in, dtype=np.float32) + np.asarray(b_h, dtype=np.float32))[
                None, :
            ].astype(ml_dtypes.bfloat16),
            (NCORES, H),
        )
    )
    # idx[core, p, t] = p*LV + veh[core*NS + t*P + p]
    base = np.arange(P, dtype=np.int64)[None, None, :] * LV  # [1, 1, P]
    idxg = (
        (base + veh.reshape(NCORES, NT, P))
        .transpose(0, 2, 1)
        .reshape(NCORES * P, NT)
        .astype(np.int32)
    )
    # memory tile slices, reshaped to the distinct out shapes
    mem_tiles = []
    m = mem.reshape(NCORES, NT, P, LV, H)
    for t in range(NT):
        s = OUT_SHAPES[t]
        mem_tiles.append(np.ascontiguousarray(m[:, t]).reshape(NCORES * s[0], *s[1:]))
    return [xtg, wtopg, whg, bsumg, idxg] + mem_tiles


_PROGRAM = None
_FN = None


def _get_fn():
    global _PROGRAM, _FN
    if _FN is None:
        _PROGRAM = build_program()
        _FN = build_dispatch(_PROGRAM)
    return _FN


def kernel(memory, veh_idx, veh_repr, cust_repr, edge_emb, W_in, b_in, W_h, b_h):
    fn = _get_fn()
    args = make_global_inputs(
        memory, veh_idx, veh_repr, cust_repr, edge_emb, W_in, b_in, W_h, b_h
    )
    outs = fn(*args)
    # outs[t] global: (NCORES*s0, ...) -> per-core (P, LV, H) tiles -> full
    full = np.empty((NCORES, NT, P, LV, H), np.float32)
    for t, o in enumerate(outs):
        full[:, t] = np.asarray(o).reshape(NCORES, P, LV, H)
    return full.reshape(N, LV, H)
